# revision 1
# baseline (speedup 1.0000x reference)
"""Trainium2 Bass kernel for multi-head causal self-attention.

Problem: X [4, 2048, 1024] fp32, Wq/Wk/Wv/Wo [1024, 1024], H=16 heads, HD=64.
reference: out = softmax_causal((X@Wq) (X@Wk)^T / 8) (X@Wv) merged @ Wo.

Sharding over 8 NeuronCores: core c handles batch b = c // 2 and head group
hg = c % 2 (8 heads each). Each core computes a partial [2048, 1024] output
(its heads' contribution through Wo's row shard); the host sums the two
partials per batch (the tensor-parallel all-reduce, done during unsharding).

Per-core dataflow (bf16 operands, fp32 PSUM accumulation):
  X^T  [1024, 2048]  bf16 via XBAR DMA-transpose straight from DRAM
  Q^T,K^T [512, 2048] = (Wq chunk).T @ X^T   (partition-chunk pc = head pair)
  V    [2048, 8, 72]  = X^T.T @ Wv, heads strided, col 64 = ones
  S^T  [128k, 512q] psum = K^T.T @ Q^T  (two heads row-packed in the PE
       array; fully-causal-masked leading columns skipped on PE)
  E^T  = exp(S^T/8 [+ diag mask]) on ACT -> bf16 SBUF; masked cols zeroed
       by GpSimd memset
  O'   [72, 512] psum = [V_h | 1 | pad].T @ E^T accumulated over k-chunks;
       row 64 = softmax denominators. Copied to SBUF immediately (frees the
       PSUM bank), then reciprocal -> gpsimd partition_broadcast -> DVE
       multiply writes normalized O^T.
  OUT  [128s, 512c] = O^T.T @ Wo chunk, accumulated over 4 o-chunks
"""

import sys

for _p in ("/opt/trn_rl_repo", "/root/.axon_site/_ro/trn_rl_repo"):
    if _p not in sys.path:
        sys.path.insert(0, _p)

import ml_dtypes
import numpy as np

import concourse.bass as bass
import concourse.mybir as mybir
import concourse.tile as tile
from concourse import bacc
from concourse.bass_utils import run_bass_kernel_spmd

F32 = mybir.dt.float32
BF16 = mybir.dt.bfloat16
EXPF = mybir.ActivationFunctionType.Exp

B, S, D, H = 4, 2048, 1024, 16
HD = D // H           # 64
HL = H // 2           # 8 heads per core
DL = HL * HD          # 512 local proj width
NEG = -30000.0        # causal mask additive value (exp underflows to 0)
VW = 72               # AV lhsT width: 64 V cols + ones col + 7 pad


def build_program(s=S, d=D, hl=HL):
    dl = hl * HD
    n_st = s // 128          # s-tiles (128 rows)
    n_dc = d // 128          # d-chunks (projection contraction)
    n_pc = dl // 128         # Q^T/K^T partition chunks (= head pairs)
    n_q = s // 512           # q-chunks
    n_k = s // 128           # k-chunks
    n_cc = d // 512          # out column chunks

    nc = bacc.Bacc("TRN2", target_bir_lowering=False, debug=False)

    X = nc.dram_tensor("X", [s, d], BF16, kind="ExternalInput")
    WQ = nc.dram_tensor("WQ", [d, dl], BF16, kind="ExternalInput")
    WK = nc.dram_tensor("WK", [d, dl], BF16, kind="ExternalInput")
    WV = nc.dram_tensor("WV", [d, dl], BF16, kind="ExternalInput")
    WO = nc.dram_tensor("WO", [dl, d], BF16, kind="ExternalInput")
    OUT = nc.dram_tensor("OUT", [s, d], F32, kind="ExternalOutput")

    with tile.TileContext(nc) as tc:
        with tc.tile_pool(name="persist", bufs=1) as persist:
            # diagonal causal mask block (keep where q >= k)
            cmask = persist.tile([128, 128], F32)
            nc.gpsimd.memset(cmask[:], 0.0)
            nc.gpsimd.affine_select(
                out=cmask[:], in_=cmask[:],
                compare_op=mybir.AluOpType.is_ge, fill=NEG,
                base=0, pattern=[[1, 128]], channel_multiplier=-1,
            )

            qt = [persist.tile([128, s], BF16, name=f"qt{i}") for i in range(n_pc)]
            kt = [persist.tile([128, s], BF16, name=f"kt{i}") for i in range(n_pc)]
            vt = [persist.tile([128, hl, VW], BF16, name=f"vt{i}") for i in range(n_st)]

            _late_cm = tc.tile_pool(name="late", bufs=1)
            late = _late_cm.__enter__()
            wo = late.tile([128, n_pc, d], BF16)
            ot = [late.tile([128, s], BF16, name=f"ot{i}") for i in range(n_pc)]

            # ---- X^T + projections (interleaved by sequence block) ----
            with (
                tc.tile_pool(name="xtp", bufs=1) as xtp,
                tc.tile_pool(name="wp", bufs=1) as wp,
                tc.tile_pool(name="pps", bufs=3, space="PSUM") as pps,
            ):
                xt = [xtp.tile([128, s], BF16, name=f"xt{i}") for i in range(n_dc)]
                wq = wp.tile([128, n_dc, dl], BF16, tag="wq", name="wq")
                wk = wp.tile([128, n_dc, dl], BF16, tag="wk", name="wk")
                wv = wp.tile([128, n_dc, dl], BF16, tag="wv", name="wv")
                # wq first: the opening projection needs it; the X^T
                # transposes already dominate the ramp
                nc.sync.dma_start(
                    wq[:], WQ.ap().rearrange("(c p) m -> p c m", p=128))
                for dc in range(n_dc):
                    nc.sync.dma_start(
                        xt[dc][:], X[:, dc * 128:(dc + 1) * 128], transpose=True)
                for wsb, wdram in ((wk, WK), (wv, WV)):
                    nc.sync.dma_start(
                        wsb[:], wdram.ap().rearrange("(c p) m -> p c m", p=128))
                nc.sync.dma_start(
                    wo[:], WO.ap().rearrange("(c p) m -> p c m", p=128))
                for nq in range(s // 512):
                    for w, dst in ((wq, qt), (wk, kt)):
                        for pc in range(n_pc):
                            ps = pps.tile([128, 512], F32, tag="ps")
                            for dc in range(n_dc):
                                nc.tensor.matmul(
                                    ps[:], w[:, dc, pc * 128:(pc + 1) * 128],
                                    xt[dc][:, nq * 512:(nq + 1) * 512],
                                    start=(dc == 0), stop=(dc == n_dc - 1))
                            nc.scalar.copy(
                                dst[pc][:, nq * 512:(nq + 1) * 512], ps[:])
                    for st in range(4 * nq, 4 * nq + 4):
                        ps = pps.tile([128, dl], F32, tag="ps")
                        for dc in range(n_dc):
                            nc.tensor.matmul(
                                ps[:], xt[dc][:, st * 128:(st + 1) * 128],
                                wv[:, dc, :],
                                start=(dc == 0), stop=(dc == n_dc - 1))
                        nc.vector.memset(vt[st][:], 1.0)
                        nc.vector.tensor_copy(
                            vt[st][:, :, 0:64],
                            ps[:].rearrange("p (h e) -> p h e", h=hl))

            # ---- attention + output projection ----
            with (
                tc.tile_pool(name="work", bufs=4) as work,
                tc.tile_pool(name="norm", bufs=3) as norm_pool,
                tc.tile_pool(name="aps", bufs=4, space="PSUM") as aps,
                tc.tile_pool(name="avps", bufs=2, space="PSUM") as avps,
                tc.tile_pool(name="ops", bufs=1, space="PSUM") as ops,
            ):
                for j in range(n_q):
                    js = slice(j * 512, (j + 1) * 512)
                    for pc in range(n_pc):
                        av = [avps.tile([VW, 512], F32, tag="av", name=f"av{j}_{pc}_0"),
                              avps.tile([VW, 512], F32, tag="av", name=f"av{j}_{pc}_1")]
                        n_i = min(4 * j + 4, n_k)
                        for i in range(n_i):
                            r = i - 4 * j
                            rs = max(r, 0) * 128   # fully-masked leading cols
                            for h in (0, 1):
                                hs = slice(64 * h, 64 * h + 64)
                                stp = aps.tile([128, 512], F32, tag="stp")
                                nc.tensor.matmul(
                                    stp[:, rs:512],
                                    kt[pc][hs, i * 128:(i + 1) * 128],
                                    qt[pc][hs, j * 512 + rs:(j + 1) * 512],
                                    start=True, stop=True,
                                    tile_position=(64 * h, 0))
                                et = work.tile([128, 512], BF16, tag="et", bufs=6)
                                if r >= 0:
                                    nc.vector.tensor_add(
                                        stp[:, rs:rs + 128], stp[:, rs:rs + 128],
                                        cmask[:])
                                    if rs:
                                        nc.gpsimd.memset(et[:, 0:rs], 0.0)
                                nc.scalar.activation(
                                    et[:, rs:512], stp[:, rs:512], EXPF, scale=0.125)
                                nc.tensor.matmul(
                                    av[h][:], vt[i][:, 2 * pc + h, :], et[:],
                                    start=(i == 0), stop=(i == n_i - 1))
                        orws = []
                        dgp = norm_pool.tile(
                            [2, 512], F32, tag="dg", bufs=4, name=f"dg{j}_{pc}")
                        for h in (0, 1):
                            # free the av bank quickly: copy O' + denominators
                            orw = norm_pool.tile(
                                [VW, 512], F32, tag="orw", bufs=4,
                                name=f"orw{j}_{pc}_{h}")
                            nc.scalar.copy(orw[:], av[h][:])
                            orws.append(orw)
                            nc.sync.dma_start(
                                dgp[h:h + 1, :], orw[64:65, :])
                        rgp = norm_pool.tile(
                            [2, 512], F32, tag="rg", bufs=4, name=f"rg{j}_{pc}")
                        nc.vector.reciprocal(rgp[:], dgp[:])
                        for h in (0, 1):
                            orw = orws[h]
                            if h == 0:
                                rsrc = rgp[0:1, :]
                            else:
                                rsb = norm_pool.tile(
                                    [1, 512], F32, tag="rsb", bufs=4,
                                    name=f"rsb{j}_{pc}")
                                nc.sync.dma_start(rsb[:], rgp[1:2, :])
                                rsrc = rsb[:]
                            bc = norm_pool.tile(
                                [128, 512], F32, tag="bc", bufs=4,
                                name=f"bc{j}_{pc}_{h}")
                            nc.gpsimd.partition_broadcast(bc[:], rsrc)
                            if h == 0:
                                nc.vector.tensor_mul(
                                    ot[pc][0:64, js], orw[0:64, :], bc[0:64, :])
                            else:
                                sc = norm_pool.tile(
                                    [64, 512], BF16, tag="sc", bufs=4,
                                    name=f"sc{j}_{pc}_{h}")
                                nc.vector.tensor_mul(
                                    sc[:], orw[0:64, :], bc[0:64, :])
                                nc.sync.dma_start(ot[pc][64:128, js], sc[:])

                    last_j = j == n_q - 1 and n_pc > 1
                    for st in range(4 * j, min(4 * j + 4, n_st)):
                        for cc in range(n_cc):
                            osb = work.tile([128, 512], F32, tag="osb", bufs=2)
                            if last_j:
                                # pairs 0..n-2 accumulate and stage to SBUF
                                # while the last pair's normalization is
                                # still in flight; final pair added after
                                ps = ops.tile([128, 512], F32, tag="outp", bufs=2)
                                for pc in range(n_pc - 1):
                                    nc.tensor.matmul(
                                        ps[:], ot[pc][:, st * 128:(st + 1) * 128],
                                        wo[:, pc, cc * 512:(cc + 1) * 512],
                                        start=(pc == 0), stop=(pc == n_pc - 2))
                                nc.vector.tensor_copy(osb[:], ps[:])
                                psb = ops.tile([128, 512], F32, tag="outp", bufs=2)
                                nc.tensor.matmul(
                                    psb[:], ot[n_pc - 1][:, st * 128:(st + 1) * 128],
                                    wo[:, n_pc - 1, cc * 512:(cc + 1) * 512],
                                    start=True, stop=True)
                                nc.vector.tensor_add(osb[:], osb[:], psb[:])
                            else:
                                ps = ops.tile([128, 512], F32, tag="outp", bufs=2)
                                for pc in range(n_pc):
                                    nc.tensor.matmul(
                                        ps[:], ot[pc][:, st * 128:(st + 1) * 128],
                                        wo[:, pc, cc * 512:(cc + 1) * 512],
                                        start=(pc == 0), stop=(pc == n_pc - 1))
                                nc.vector.tensor_copy(osb[:], ps[:])
                            nc.sync.dma_start(
                                OUT[st * 128:(st + 1) * 128,
                                    cc * 512:(cc + 1) * 512],
                                osb[:])

            _late_cm.__exit__(None, None, None)

    nc.compile()
    return nc


_NC_CACHE = {}


def _get_program():
    key = (S, D, HL)
    if key not in _NC_CACHE:
        _NC_CACHE[key] = build_program()
    return _NC_CACHE[key]


def _bf16(a):
    return np.ascontiguousarray(a.astype(ml_dtypes.bfloat16))


def make_in_maps(X, Wq, Wk, Wv, Wo):
    in_maps = []
    for c in range(8):
        b, hg = c // 2, c % 2
        cs = slice(hg * DL, hg * DL + DL)
        in_maps.append({
            "X": _bf16(X[b]),
            "WQ": _bf16(Wq[:, cs]),
            "WK": _bf16(Wk[:, cs]),
            "WV": _bf16(Wv[:, cs]),
            "WO": _bf16(Wo[cs, :]),
        })
    return in_maps


def gather_out(results):
    out = np.empty((B, S, D), dtype=np.float32)
    for b in range(B):
        out[b] = results[2 * b]["OUT"] + results[2 * b + 1]["OUT"]
    return out


def kernel(X, Wq, Wk, Wv, Wo):
    X = np.asarray(X, dtype=np.float32)
    Wq = np.asarray(Wq, dtype=np.float32)
    Wk = np.asarray(Wk, dtype=np.float32)
    Wv = np.asarray(Wv, dtype=np.float32)
    Wo = np.asarray(Wo, dtype=np.float32)

    nc = _get_program()
    in_maps = make_in_maps(X, Wq, Wk, Wv, Wo)
    res = run_bass_kernel_spmd(nc, in_maps, list(range(8)), trace=False)
    return gather_out(res.results)


if __name__ == "__main__":
    rng = np.random.default_rng(0)
    scale = 1.0 / np.sqrt(D)
    inputs = {
        "X": rng.standard_normal((B, S, D), dtype=np.float32),
        "Wq": rng.standard_normal((D, D), dtype=np.float32) * scale,
        "Wk": rng.standard_normal((D, D), dtype=np.float32) * scale,
        "Wv": rng.standard_normal((D, D), dtype=np.float32) * scale,
        "Wo": rng.standard_normal((D, D), dtype=np.float32) * scale,
    }
    out = kernel(**inputs)
    print("kernel output shape:", out.shape)



# revision 2
# speedup vs baseline: 1.2839x; 1.2839x over previous
"""Trainium2 Bass kernel for multi-head causal self-attention.

Problem: X [4, 2048, 1024] fp32, Wq/Wk/Wv/Wo [1024, 1024], H=16 heads, HD=64.
reference: out = softmax_causal((X@Wq) (X@Wk)^T / 8) (X@Wv) merged @ Wo.

Sharding over 8 NeuronCores: core c handles batch b = c // 2 and head group
hg = c % 2 (8 heads each). Each core computes a partial [2048, 1024] output
(its heads' contribution through Wo's row shard); the host sums the two
partials per batch (the tensor-parallel all-reduce, done during unsharding).

Per-core dataflow (bf16 operands, fp32 PSUM accumulation), software-pipelined
so the PE never starves (keeps the HAM clock-gate warm):

  ramp     X^T via DMA-transpose on BOTH HWDGE rings (sync + scalar);
           Q^T/K^T/V projections for seq chunk 0 pipelined per d-chunk.
  stage j  attention for q-chunk j (512 q rows x all k-blocks <= diag):
             S^T pair [128k, 2x512q] psum (2 banks): both heads' QK^T
               matmuls emitted adjacently with tile_position row packing so
               they run CONCURRENTLY in the PE array (64-contraction each).
             exp on ACT as ONE [128, 2, 512-rs] instruction per k-block
               (both heads), bf16 out; fully-masked leading cols skipped,
               diagonal blocks get a cmask add (DVE) pre-exp.
             AV accumulated over k-blocks into [72, 512] psum per head;
               col 64 of V = ones => row 64 = softmax denominators.
           Interleaved as PE filler: projections for chunk j+1 (stages 0-2)
           and the output projection for chunks 0..2 (stage 3), so the PE
           stream stays dense while ACT works through the exps.
  norm     reciprocal_approx_fast on the denominators (5x faster than the
           iterative divide), gpsimd partition_broadcast, DVE multiply.
  out      OUT [128s, 512c] = O^T.T @ Wo accumulated over 4 head-pair
           chunks; last chunk's final head-pair contribution added
           separately so the tail doesn't serialize.
"""

import itertools
import sys

for _p in ("/opt/trn_rl_repo", "/root/.axon_site/_ro/trn_rl_repo"):
    if _p not in sys.path:
        sys.path.insert(0, _p)

import ml_dtypes
import numpy as np

import concourse.bass as bass
import concourse.mybir as mybir
import concourse.tile as tile
from concourse import bacc
from concourse.bass_utils import run_bass_kernel_spmd

F32 = mybir.dt.float32
BF16 = mybir.dt.bfloat16
EXPF = mybir.ActivationFunctionType.Exp

B, S, D, H = 4, 2048, 1024, 16
HD = D // H           # 64
HL = H // 2           # 8 heads per core
DL = HL * HD          # 512 local proj width
NEG = -30000.0        # causal mask additive value (exp underflows to 0)
VW = 72               # AV lhsT width: 64 V cols + ones col + 7 pad


class _Filler:
    """Interleave a generator of PE work quanta at a fractional rate."""

    def __init__(self, gens):
        self.it = itertools.chain(*gens)
        self.frac = 0.0
        self.done = False

    def pump(self, amount):
        if self.done:
            return
        self.frac += amount
        while self.frac >= 1.0:
            try:
                next(self.it)
            except StopIteration:
                self.done = True
                return
            self.frac -= 1.0

    def drain(self):
        for _ in self.it:
            pass
        self.done = True


def build_program(s=S, d=D, hl=HL):
    dl = hl * HD
    n_st = s // 128          # 16 s-tiles (128 rows)
    n_dc = d // 128          # 8 d-chunks (projection contraction)
    n_pc = dl // 128         # 4 head-pair chunks
    n_q = s // 512           # 4 q-chunks
    n_k = s // 128           # 16 k-blocks
    n_cc = d // 512          # 2 out column chunks

    nc = bacc.Bacc("TRN2", target_bir_lowering=False, debug=False)

    X = nc.dram_tensor("X", [s, d], BF16, kind="ExternalInput")
    WQ = nc.dram_tensor("WQ", [d, dl], BF16, kind="ExternalInput")
    WK = nc.dram_tensor("WK", [d, dl], BF16, kind="ExternalInput")
    WV = nc.dram_tensor("WV", [d, dl], BF16, kind="ExternalInput")
    WO = nc.dram_tensor("WO", [dl, d], BF16, kind="ExternalInput")
    OUT = nc.dram_tensor("OUT", [s, d], F32, kind="ExternalOutput")

    with tile.TileContext(nc) as tc:
        with tc.tile_pool(name="persist", bufs=1) as persist:
            # diagonal causal mask block x2 (keep where q >= k), one copy
            # per head so a single DVE add masks both heads' diag blocks
            cmask2 = persist.tile([128, 2, 128], F32, name="cmask2")
            nc.gpsimd.memset(cmask2[:], 0.0)
            for hb in (0, 1):
                nc.gpsimd.affine_select(
                    out=cmask2[:, hb, :], in_=cmask2[:, hb, :],
                    compare_op=mybir.AluOpType.is_ge, fill=NEG,
                    base=0, pattern=[[1, 128]], channel_multiplier=-1,
                )

            xt = [persist.tile([128, s], BF16, name=f"xt{i}") for i in range(n_dc)]
            qt = [persist.tile([128, s], BF16, name=f"qt{i}") for i in range(n_pc)]
            kt = [persist.tile([128, s], BF16, name=f"kt{i}") for i in range(n_pc)]
            vt = [persist.tile([128, hl, VW], BF16, name=f"vt{i}") for i in range(n_st)]
            ot = [persist.tile([128, s], BF16, name=f"ot{i}") for i in range(n_pc)]
            wq = persist.tile([128, n_dc, dl], BF16, name="wq")
            wk = persist.tile([128, n_dc, dl], BF16, name="wk")
            wv = persist.tile([128, n_dc, dl], BF16, name="wv")
            wo = persist.tile([128, n_pc, d], BF16, name="wo")

            # DMA kickoff on both HWDGE rings (sync + scalar) so the X^T
            # transposes overlap: evens on sync, weights + odds on scalar.
            nc.scalar.dma_start(
                wq[:], WQ.ap().rearrange("(c p) m -> p c m", p=128))
            nc.scalar.dma_start(
                wk[:], WK.ap().rearrange("(c p) m -> p c m", p=128))
            for dc in (0, 2, 4, 6):
                nc.sync.dma_start(
                    xt[dc][:], X[:, dc * 128:(dc + 1) * 128], transpose=True)
            for dc in (1, 3, 5, 7):
                nc.scalar.dma_start(
                    xt[dc][:], X[:, dc * 128:(dc + 1) * 128], transpose=True)
            nc.sync.dma_start(
                wv[:], WV.ap().rearrange("(c p) m -> p c m", p=128))
            nc.sync.dma_start(
                wo[:], WO.ap().rearrange("(c p) m -> p c m", p=128))

            with (
                tc.tile_pool(name="ppp", bufs=2, space="PSUM") as ppp,
                tc.tile_pool(name="stpp", bufs=2, space="PSUM") as stpp,
                tc.tile_pool(name="avp", bufs=2, space="PSUM") as avp,
                tc.tile_pool(name="work", bufs=4) as work,
                tc.tile_pool(name="osbp", bufs=8) as osbp,
            ):
                def gen_proj(nq):
                    """Projection of seq chunk nq; yields per PE quantum."""
                    dcs = [0, 2, 4, 6, 1, 3, 5, 7] if nq == 0 else list(range(n_dc))
                    qs = slice(nq * 512, (nq + 1) * 512)
                    for pc in range(n_pc):
                        for w, dst, cpy in ((wq, qt, "act"), (wk, kt, "dve")):
                            ps = ppp.tile([128, 512], F32, tag="pp",
                                          name=f"psp{nq}_{pc}")
                            for ii, dc in enumerate(dcs):
                                nc.tensor.matmul(
                                    ps[:], w[:, dc, pc * 128:(pc + 1) * 128],
                                    xt[dc][:, qs],
                                    start=(ii == 0), stop=(ii == n_dc - 1))
                                yield
                            if cpy == "act":
                                nc.scalar.copy(dst[pc][:, qs], ps[:])
                            else:
                                nc.vector.tensor_copy(dst[pc][:, qs], ps[:])
                            yield
                    for st in range(4 * nq, 4 * nq + 4):
                        ps = ppp.tile([128, dl], F32, tag="pp",
                                      name=f"psv{nq}_{st}")
                        for ii, dc in enumerate(dcs):
                            nc.tensor.matmul(
                                ps[:], xt[dc][:, st * 128:(st + 1) * 128],
                                wv[:, dc, :],
                                start=(ii == 0), stop=(ii == n_dc - 1))
                            yield
                        nc.gpsimd.memset(vt[st][:], 1.0)
                        nc.vector.tensor_copy(
                            vt[st][:, :, 0:64],
                            ps[:].rearrange("p (h e) -> p h e", h=hl))
                        yield

                def gen_outproj(j, stash=None):
                    """Output projection for seq chunk j. If stash is given,
                    only head-pairs 0..n_pc-2 are accumulated and the psum
                    copy is stashed for a later finisher (last pair added
                    once its normalize lands)."""
                    npc = n_pc - 1 if stash is not None else n_pc
                    for st in range(4 * j, 4 * j + 4):
                        for cc in range(n_cc):
                            ps = ppp.tile([128, 512], F32, tag="pp",
                                          name=f"pso{st}_{cc}")
                            for pc in range(npc):
                                nc.tensor.matmul(
                                    ps[:], ot[pc][:, st * 128:(st + 1) * 128],
                                    wo[:, pc, cc * 512:(cc + 1) * 512],
                                    start=(pc == 0), stop=(pc == npc - 1))
                                yield
                            osb = osbp.tile([128, 512], F32, tag="osb",
                                            name=f"osb{st}_{cc}")
                            nc.vector.tensor_copy(osb[:], ps[:])
                            yield
                            if stash is not None:
                                stash.append((st, cc, osb))
                            else:
                                nc.sync.dma_start(
                                    OUT[st * 128:(st + 1) * 128,
                                        cc * 512:(cc + 1) * 512],
                                    osb[:])
                                yield

                def attn_unit(j, pc, fillers):
                    js = slice(j * 512, (j + 1) * 512)
                    n_i = min(4 * j + 4, n_k)
                    av = [avp.tile([VW, 512], F32, tag="av",
                                   name=f"av{j}_{pc}_{h}") for h in (0, 1)]
                    for i in range(n_i):
                        r = i - 4 * j
                        rs = max(r, 0) * 128   # fully-masked leading cols
                        stp = stpp.tile([128, 2, 512], F32, tag="stp",
                                        name=f"stp{j}_{pc}_{i}")
                        for h in (0, 1):
                            hs = slice(64 * h, 64 * h + 64)
                            nc.tensor.matmul(
                                stp[:, h, rs:512],
                                kt[pc][hs, i * 128:(i + 1) * 128],
                                qt[pc][hs, j * 512 + rs:(j + 1) * 512],
                                start=True, stop=True,
                                tile_position=(64 * h, 0))
                        if r >= 0:
                            nc.vector.tensor_add(
                                stp[:, :, rs:rs + 128], stp[:, :, rs:rs + 128],
                                cmask2[:])
                        et = work.tile([128, 2, 512], BF16, tag="et", bufs=4,
                                       name=f"et{j}_{pc}_{i}")
                        nc.scalar.activation(
                            et[:, :, rs:512], stp[:, :, rs:512], EXPF,
                            scale=0.125)
                        for h in (0, 1):
                            nc.tensor.matmul(
                                av[h][:, rs:512], vt[i][:, 2 * pc + h, :],
                                et[:, h, rs:512],
                                start=(i == 0), stop=(i == n_i - 1))
                        for flr, rate in fillers:
                            flr.pump(rate)

                    # normalize: denominators live in av row 64
                    orws = []
                    dgp = work.tile([2, 512], F32, tag="dg", bufs=4,
                                    name=f"dg{j}_{pc}")
                    for h in (0, 1):
                        orw = work.tile([VW, 512], F32, tag="orw", bufs=4,
                                        name=f"orw{j}_{pc}_{h}")
                        nc.vector.tensor_copy(orw[:], av[h][:])
                        orws.append(orw)
                        nc.sync.dma_start(dgp[h:h + 1, :], orw[64:65, :])
                    rgp = work.tile([2, 512], F32, tag="rg", bufs=4,
                                    name=f"rg{j}_{pc}")
                    nc.vector.reciprocal_approx_fast(rgp[:], dgp[:])
                    for h in (0, 1):
                        if h == 0:
                            rsrc = rgp[0:1, :]
                        else:
                            rsb = work.tile([1, 512], F32, tag="rsb", bufs=4,
                                            name=f"rsb{j}_{pc}")
                            nc.sync.dma_start(rsb[:], rgp[1:2, :])
                            rsrc = rsb[:]
                        bc = work.tile([64, 512], F32, tag="bc", bufs=4,
                                       name=f"bc{j}_{pc}_{h}")
                        nc.gpsimd.partition_broadcast(bc[:], rsrc)
                        if h == 0:
                            nc.vector.tensor_mul(
                                ot[pc][0:64, js], orws[0][0:64, :], bc[:])
                        else:
                            sc = work.tile([64, 512], BF16, tag="sc", bufs=4,
                                           name=f"sc{j}_{pc}")
                            nc.vector.tensor_mul(sc[:], orws[1][0:64, :], bc[:])
                            nc.sync.dma_start(ot[pc][64:128, js], sc[:])

                # ---- ramp: projections for chunk 0, dc-pipelined ----
                for _ in gen_proj(0):
                    pass

                # ---- pipelined stages ----
                stash = []
                for j in range(n_q):
                    if j < n_q - 1:
                        filler = _Filler([gen_proj(j + 1)])
                        rate = {0: 7.0, 1: 3.6, 2: 2.4}[j]
                    else:
                        filler = _Filler([gen_outproj(0), gen_outproj(1),
                                          gen_outproj(2)])
                        rate = 2.3
                    for pc in range(n_pc):
                        fillers = [(filler, rate)]
                        if j == n_q - 1 and pc == n_pc - 1:
                            part1 = _Filler([gen_outproj(3, stash=stash)])
                            fillers.append((part1, 2.0))
                        attn_unit(j, pc, fillers)
                        if j == n_q - 1 and pc == n_pc - 1:
                            part1.drain()
                    filler.drain()

                # ---- finisher: add last head-pair into chunk-3 outputs ----
                for st, cc, osb in stash:
                    psb = ppp.tile([128, 512], F32, tag="pp",
                                   name=f"psb{st}_{cc}")
                    nc.tensor.matmul(
                        psb[:], ot[n_pc - 1][:, st * 128:(st + 1) * 128],
                        wo[:, n_pc - 1, cc * 512:(cc + 1) * 512],
                        start=True, stop=True)
                    nc.vector.tensor_add(osb[:], osb[:], psb[:])
                    nc.sync.dma_start(
                        OUT[st * 128:(st + 1) * 128,
                            cc * 512:(cc + 1) * 512],
                        osb[:])

    nc.compile()
    return nc


_NC_CACHE = {}


def _get_program():
    key = (S, D, HL)
    if key not in _NC_CACHE:
        _NC_CACHE[key] = build_program()
    return _NC_CACHE[key]


def _bf16(a):
    return np.ascontiguousarray(a.astype(ml_dtypes.bfloat16))


def make_in_maps(X, Wq, Wk, Wv, Wo):
    in_maps = []
    for c in range(8):
        b, hg = c // 2, c % 2
        cs = slice(hg * DL, hg * DL + DL)
        in_maps.append({
            "X": _bf16(X[b]),
            "WQ": _bf16(Wq[:, cs]),
            "WK": _bf16(Wk[:, cs]),
            "WV": _bf16(Wv[:, cs]),
            "WO": _bf16(Wo[cs, :]),
        })
    return in_maps


def gather_out(results):
    out = np.empty((B, S, D), dtype=np.float32)
    for b in range(B):
        out[b] = results[2 * b]["OUT"] + results[2 * b + 1]["OUT"]
    return out


def kernel(X, Wq, Wk, Wv, Wo):
    X = np.asarray(X, dtype=np.float32)
    Wq = np.asarray(Wq, dtype=np.float32)
    Wk = np.asarray(Wk, dtype=np.float32)
    Wv = np.asarray(Wv, dtype=np.float32)
    Wo = np.asarray(Wo, dtype=np.float32)

    nc = _get_program()
    in_maps = make_in_maps(X, Wq, Wk, Wv, Wo)
    res = run_bass_kernel_spmd(nc, in_maps, list(range(8)), trace=False)
    return gather_out(res.results)


if __name__ == "__main__":
    rng = np.random.default_rng(0)
    scale = 1.0 / np.sqrt(D)
    inputs = {
        "X": rng.standard_normal((B, S, D), dtype=np.float32),
        "Wq": rng.standard_normal((D, D), dtype=np.float32) * scale,
        "Wk": rng.standard_normal((D, D), dtype=np.float32) * scale,
        "Wv": rng.standard_normal((D, D), dtype=np.float32) * scale,
        "Wo": rng.standard_normal((D, D), dtype=np.float32) * scale,
    }
    out = kernel(**inputs)
    print("kernel output shape:", out.shape)


# revision 9
# speedup vs baseline: 1.3518x; 1.0529x over previous
"""Trainium2 Bass kernel for multi-head causal self-attention.

Problem: X [4, 2048, 1024] fp32, Wq/Wk/Wv/Wo [1024, 1024], H=16 heads, HD=64.
reference: out = softmax_causal((X@Wq) (X@Wk)^T / 8) (X@Wv) merged @ Wo.

Sharding over 8 NeuronCores: core c handles batch b = c // 2 and head group
hg = c % 2 (8 heads each). Each core computes a partial [2048, 1024] output
(its heads' contribution through Wo's row shard); the host sums the two
partials per batch (the tensor-parallel all-reduce, done during unsharding).

Per-core dataflow (bf16 operands, fp32 PSUM accumulation), software-pipelined
so the PE never starves (keeps the HAM clock-gate warm):

  ramp     X^T via DMA-transpose on BOTH HWDGE rings (sync + scalar);
           Q^T/K^T/V projections for seq chunk 0 pipelined per d-chunk.
  stage j  attention for q-chunk j (512 q rows x all k-blocks <= diag):
             S^T pair [128k, 2x512q] psum (2 banks): both heads' QK^T
               matmuls emitted adjacently with tile_position row packing so
               they run CONCURRENTLY in the PE array (64-contraction each).
             exp on ACT as ONE [128, 2, 512-rs] instruction per k-block
               (both heads), bf16 out; fully-masked leading cols skipped,
               diagonal blocks get a cmask add (DVE) pre-exp.
             AV accumulated over k-blocks into [72, 512] psum per head;
               col 64 of V = ones => row 64 = softmax denominators.
           Interleaved as PE filler: projections for chunk j+1 (stages 0-2)
           and the output projection for chunks 0..2 (stage 3), so the PE
           stream stays dense while ACT works through the exps.
  norm     reciprocal_approx_fast on the denominators (5x faster than the
           iterative divide), gpsimd partition_broadcast, DVE multiply.
  out      OUT [128s, 512c] = O^T.T @ Wo accumulated over 4 head-pair
           chunks; last chunk's final head-pair contribution added
           separately so the tail doesn't serialize.
"""

import itertools
import sys

for _p in ("/opt/trn_rl_repo", "/root/.axon_site/_ro/trn_rl_repo"):
    if _p not in sys.path:
        sys.path.insert(0, _p)

import ml_dtypes
import numpy as np

import concourse.bass as bass
import concourse.mybir as mybir
import concourse.tile as tile
from concourse import bacc
from concourse.bass_utils import run_bass_kernel_spmd

F32 = mybir.dt.float32
BF16 = mybir.dt.bfloat16
EXPF = mybir.ActivationFunctionType.Exp

B, S, D, H = 4, 2048, 1024, 16
HD = D // H           # 64
HL = H // 2           # 8 heads per core
DL = HL * HD          # 512 local proj width
NEG = -30000.0        # causal mask additive value (exp underflows to 0)
VW = 72               # AV lhsT width: 64 V cols + ones col + 7 pad


class _Filler:
    """Interleave a generator of PE work quanta at a fractional rate."""

    def __init__(self, gens):
        self.it = itertools.chain(*gens)
        self.frac = 0.0
        self.done = False

    def pump(self, amount):
        if self.done:
            return
        self.frac += amount
        while self.frac >= 1.0:
            try:
                next(self.it)
            except StopIteration:
                self.done = True
                return
            self.frac -= 1.0

    def drain(self):
        for _ in self.it:
            pass
        self.done = True


def build_program(s=S, d=D, hl=HL):
    dl = hl * HD
    n_st = s // 128          # 16 s-tiles (128 rows)
    n_dc = d // 128          # 8 d-chunks (projection contraction)
    n_pc = dl // 128         # 4 head-pair chunks
    n_q = s // 512           # 4 q-chunks
    n_k = s // 128           # 16 k-blocks
    n_cc = d // 512          # 2 out column chunks

    nc = bacc.Bacc("TRN2", target_bir_lowering=False, debug=False)

    X = nc.dram_tensor("X", [s, d], BF16, kind="ExternalInput")
    WQ = nc.dram_tensor("WQ", [d, dl], BF16, kind="ExternalInput")
    WK = nc.dram_tensor("WK", [d, dl], BF16, kind="ExternalInput")
    WV = nc.dram_tensor("WV", [d, dl], BF16, kind="ExternalInput")
    WO = nc.dram_tensor("WO", [dl, d], BF16, kind="ExternalInput")
    OUT = nc.dram_tensor("OUT", [s, d], F32, kind="ExternalOutput")

    with tile.TileContext(nc) as tc:
        with tc.tile_pool(name="persist", bufs=1) as persist:
            # diagonal causal mask block x2 (keep where q >= k), one copy
            # per head so a single DVE add masks both heads' diag blocks
            cmask2 = persist.tile([128, 2, 128], F32, name="cmask2")
            nc.gpsimd.memset(cmask2[:], 0.0)
            for hb in (0, 1):
                nc.gpsimd.affine_select(
                    out=cmask2[:, hb, :], in_=cmask2[:, hb, :],
                    compare_op=mybir.AluOpType.is_ge, fill=NEG,
                    base=0, pattern=[[1, 128]], channel_multiplier=-1,
                )

            # X^T in chunk-major layout: xt[p, nq, dc, m] = X^T[dc*128+p,
            # nq*512+m]. Each seq-quarter of X is one CONTIGUOUS DMA
            # transpose writing one contiguous SBUF region — DMA transposes
            # serialize globally against all other DMAs (HW deadlock guard),
            # so fewer/bigger transposes shorten the ramp chain.
            xt = persist.tile([128, n_q, n_dc, 512], BF16, name="xt")
            qt = [persist.tile([128, s], BF16, name=f"qt{i}") for i in range(n_pc)]
            kt = [persist.tile([128, s], BF16, name=f"kt{i}") for i in range(n_pc)]
            vt = [persist.tile([128, hl, VW], BF16, name=f"vt{i}") for i in range(n_st)]
            ot = [persist.tile([128, s], BF16, name=f"ot{i}") for i in range(n_pc)]
            wq = persist.tile([128, n_dc, dl], BF16, name="wq")
            wk = persist.tile([128, n_dc, dl], BF16, name="wk")
            wv = persist.tile([128, n_dc, dl], BF16, name="wv")
            wo = persist.tile([128, n_pc, d], BF16, name="wo")

            # DMA kickoff: weights first (plain DMAs pipeline), then the 4
            # quarter transposes (each serializes against in-flight DMAs).
            nc.scalar.dma_start(
                wq[:], WQ.ap().rearrange("(c p) m -> p c m", p=128))
            nc.scalar.dma_start(
                wk[:], WK.ap().rearrange("(c p) m -> p c m", p=128))
            nc.scalar.dma_start(
                wv[:], WV.ap().rearrange("(c p) m -> p c m", p=128))
            for nq in range(n_q):
                nc.sync.dma_start(
                    xt[:, nq], X[nq * 512:(nq + 1) * 512, :], transpose=True)
            nc.scalar.dma_start(
                wo[:], WO.ap().rearrange("(c p) m -> p c m", p=128))

            with (
                tc.tile_pool(name="ppp", bufs=2, space="PSUM") as ppp,
                tc.tile_pool(name="stpp", bufs=2, space="PSUM") as stpp,
                tc.tile_pool(name="avp", bufs=2, space="PSUM") as avp,
                tc.tile_pool(name="work", bufs=4) as work,
                tc.tile_pool(name="osbp", bufs=8) as osbp,
            ):
                def gen_proj(nq):
                    """Projection of seq chunk nq; yields per PE quantum.
                    Order Q-pc, K-pc, V-st interleaved so the first
                    attention unit's inputs land earliest."""
                    for pc in range(n_pc):
                        for w, dst, cpy in ((wq, qt, "act"), (wk, kt, "dve")):
                            ps = ppp.tile([128, 512], F32, tag="pp",
                                          name=f"psp{nq}_{pc}")
                            for dc in range(n_dc):
                                nc.tensor.matmul(
                                    ps[:], w[:, dc, pc * 128:(pc + 1) * 128],
                                    xt[:, nq, dc, :],
                                    start=(dc == 0), stop=(dc == n_dc - 1))
                                yield
                            qs = slice(nq * 512, (nq + 1) * 512)
                            if cpy == "act":
                                nc.scalar.copy(dst[pc][:, qs], ps[:])
                            else:
                                nc.vector.tensor_copy(dst[pc][:, qs], ps[:])
                            yield
                        st = 4 * nq + pc
                        ps = ppp.tile([128, dl], F32, tag="pp",
                                      name=f"psv{nq}_{st}")
                        for dc in range(n_dc):
                            nc.tensor.matmul(
                                ps[:], xt[:, nq, dc, pc * 128:(pc + 1) * 128],
                                wv[:, dc, :],
                                start=(dc == 0), stop=(dc == n_dc - 1))
                            yield
                        nc.gpsimd.memset(vt[st][:], 1.0)
                        nc.vector.tensor_copy(
                            vt[st][:, :, 0:64],
                            ps[:].rearrange("p (h e) -> p h e", h=hl))
                        yield

                def gen_outproj(j, stash=None):
                    """Output projection for seq chunk j. If stash is given,
                    only head-pairs 0..n_pc-2 are accumulated and the psum
                    copy is stashed for a later finisher (last pair added
                    once its normalize lands)."""
                    npc = n_pc - 1 if stash is not None else n_pc
                    for st in range(4 * j, 4 * j + 4):
                        for cc in range(n_cc):
                            ps = ppp.tile([128, 512], F32, tag="pp",
                                          name=f"pso{st}_{cc}")
                            for pc in range(npc):
                                nc.tensor.matmul(
                                    ps[:], ot[pc][:, st * 128:(st + 1) * 128],
                                    wo[:, pc, cc * 512:(cc + 1) * 512],
                                    start=(pc == 0), stop=(pc == npc - 1))
                                yield
                            osb = osbp.tile([128, 512], F32, tag="osb",
                                            name=f"osb{st}_{cc}")
                            nc.vector.tensor_copy(osb[:], ps[:])
                            yield
                            if stash is not None:
                                stash.append((st, cc, osb))
                            else:
                                # alternate rings so OUT writes don't back up
                                eng = nc.sync if (st + cc) % 2 == 0 else nc.scalar
                                eng.dma_start(
                                    OUT[st * 128:(st + 1) * 128,
                                        cc * 512:(cc + 1) * 512],
                                    osb[:])
                                yield

                def attn_unit(j, pc, fillers):
                    js = slice(j * 512, (j + 1) * 512)
                    n_i = min(4 * j + 4, n_k)
                    av = [avp.tile([VW, 512], F32, tag="av",
                                   name=f"av{j}_{pc}_{h}") for h in (0, 1)]
                    for i in range(n_i):
                        r = i - 4 * j
                        rs = max(r, 0) * 128   # fully-masked leading cols
                        stp = stpp.tile([128, 2, 512], F32, tag="stp",
                                        name=f"stp{j}_{pc}_{i}")
                        for h in (0, 1):
                            hs = slice(64 * h, 64 * h + 64)
                            nc.tensor.matmul(
                                stp[:, h, rs:512],
                                kt[pc][hs, i * 128:(i + 1) * 128],
                                qt[pc][hs, j * 512 + rs:(j + 1) * 512],
                                start=True, stop=True,
                                tile_position=(64 * h, 0))
                        if r >= 0:
                            nc.vector.tensor_add(
                                stp[:, :, rs:rs + 128], stp[:, :, rs:rs + 128],
                                cmask2[:])
                        et = work.tile([128, 2, 512], BF16, tag="et", bufs=6,
                                       name=f"et{j}_{pc}_{i}")
                        nc.scalar.activation(
                            et[:, :, rs:512], stp[:, :, rs:512], EXPF,
                            scale=0.125)
                        for h in (0, 1):
                            nc.tensor.matmul(
                                av[h][:, rs:512], vt[i][:, 2 * pc + h, :],
                                et[:, h, rs:512],
                                start=(i == 0), stop=(i == n_i - 1))
                        for flr, rate in fillers:
                            flr.pump(rate)

                    # normalize: denominators live in av row 64. Per-head
                    # independent chains: copy O'+denom to SBUF, DMA the
                    # denom row to partition 0, approx-reciprocal, gpsimd
                    # broadcast, DVE multiply (h1 DMA-shifts to rows 64-127).
                    for h in (0, 1):
                        orw = work.tile([VW, 512], F32, tag="orw", bufs=4,
                                        name=f"orw{j}_{pc}_{h}")
                        nc.vector.tensor_copy(orw[:], av[h][:])
                        dgp = work.tile([1, 512], F32, tag=f"dg{h}", bufs=3,
                                        name=f"dg{j}_{pc}_{h}")
                        nc.sync.dma_start(dgp[:], orw[64:65, :])
                        rgp = work.tile([1, 512], F32, tag=f"rg{h}", bufs=3,
                                        name=f"rg{j}_{pc}_{h}")
                        nc.vector.reciprocal_approx_fast(rgp[:], dgp[:])
                        bc = work.tile([64, 512], F32, tag=f"bc{h}", bufs=3,
                                       name=f"bc{j}_{pc}_{h}")
                        nc.gpsimd.partition_broadcast(bc[:], rgp[:])
                        if h == 0:
                            nc.vector.tensor_mul(
                                ot[pc][0:64, js], orw[0:64, :], bc[:])
                        else:
                            sc = work.tile([64, 512], BF16, tag="sc", bufs=3,
                                           name=f"sc{j}_{pc}")
                            nc.vector.tensor_mul(sc[:], orw[0:64, :], bc[:])
                            nc.sync.dma_start(ot[pc][64:128, js], sc[:])

                # ---- ramp: projections for chunk 0, dc-pipelined ----
                for _ in gen_proj(0):
                    pass

                # ---- pipelined stages ----
                stash = []
                for j in range(n_q):
                    if j < n_q - 1:
                        filler = _Filler([gen_proj(j + 1)])
                        rate = {0: 7.0, 1: 3.6, 2: 2.4}[j]
                    else:
                        filler = _Filler([gen_outproj(0), gen_outproj(1),
                                          gen_outproj(2)])
                        rate = 2.3
                    for pc in range(n_pc):
                        fillers = [(filler, rate)]
                        if j == n_q - 1 and pc == n_pc - 1:
                            part1 = _Filler([gen_outproj(3, stash=stash)])
                            fillers.append((part1, 2.0))
                        attn_unit(j, pc, fillers)
                        if j == n_q - 1 and pc == n_pc - 1:
                            part1.drain()
                    filler.drain()

                # ---- finisher: add last head-pair into chunk-3 outputs ----
                for st, cc, osb in stash:
                    psb = ppp.tile([128, 512], F32, tag="pp",
                                   name=f"psb{st}_{cc}")
                    nc.tensor.matmul(
                        psb[:], ot[n_pc - 1][:, st * 128:(st + 1) * 128],
                        wo[:, n_pc - 1, cc * 512:(cc + 1) * 512],
                        start=True, stop=True)
                    nc.vector.tensor_add(osb[:], osb[:], psb[:])
                    eng = nc.sync if (st + cc) % 2 == 0 else nc.scalar
                    eng.dma_start(
                        OUT[st * 128:(st + 1) * 128,
                            cc * 512:(cc + 1) * 512],
                        osb[:])

    nc.compile()
    return nc


_NC_CACHE = {}


def _get_program():
    key = (S, D, HL)
    if key not in _NC_CACHE:
        _NC_CACHE[key] = build_program()
    return _NC_CACHE[key]


def _bf16(a):
    return np.ascontiguousarray(a.astype(ml_dtypes.bfloat16))


def make_in_maps(X, Wq, Wk, Wv, Wo):
    in_maps = []
    for c in range(8):
        b, hg = c // 2, c % 2
        cs = slice(hg * DL, hg * DL + DL)
        in_maps.append({
            "X": _bf16(X[b]),
            "WQ": _bf16(Wq[:, cs]),
            "WK": _bf16(Wk[:, cs]),
            "WV": _bf16(Wv[:, cs]),
            "WO": _bf16(Wo[cs, :]),
        })
    return in_maps


def gather_out(results):
    out = np.empty((B, S, D), dtype=np.float32)
    for b in range(B):
        out[b] = results[2 * b]["OUT"] + results[2 * b + 1]["OUT"]
    return out


def kernel(X, Wq, Wk, Wv, Wo):
    X = np.asarray(X, dtype=np.float32)
    Wq = np.asarray(Wq, dtype=np.float32)
    Wk = np.asarray(Wk, dtype=np.float32)
    Wv = np.asarray(Wv, dtype=np.float32)
    Wo = np.asarray(Wo, dtype=np.float32)

    nc = _get_program()
    in_maps = make_in_maps(X, Wq, Wk, Wv, Wo)
    res = run_bass_kernel_spmd(nc, in_maps, list(range(8)), trace=False)
    return gather_out(res.results)


if __name__ == "__main__":
    rng = np.random.default_rng(0)
    scale = 1.0 / np.sqrt(D)
    inputs = {
        "X": rng.standard_normal((B, S, D), dtype=np.float32),
        "Wq": rng.standard_normal((D, D), dtype=np.float32) * scale,
        "Wk": rng.standard_normal((D, D), dtype=np.float32) * scale,
        "Wv": rng.standard_normal((D, D), dtype=np.float32) * scale,
        "Wo": rng.standard_normal((D, D), dtype=np.float32) * scale,
    }
    out = kernel(**inputs)
    print("kernel output shape:", out.shape)


# revision 16
# speedup vs baseline: 1.3974x; 1.0337x over previous
"""Trainium2 Bass kernel for multi-head causal self-attention.

Problem: X [4, 2048, 1024] fp32, Wq/Wk/Wv/Wo [1024, 1024], H=16 heads, HD=64.
reference: out = softmax_causal((X@Wq) (X@Wk)^T / 8) (X@Wv) merged @ Wo.

Sharding over 8 NeuronCores: core c handles batch b = c // 2 and head group
hg = c % 2 (8 heads each). Each core computes a partial [2048, 1024] output
(its heads' contribution through Wo's row shard); the host sums the two
partials per batch (the tensor-parallel all-reduce, done during unsharding).

Per-core dataflow (bf16 operands, fp32 PSUM accumulation), software-pipelined
so the PE never starves (keeps the HAM clock-gate warm):

  ramp     X^T via DMA-transpose on BOTH HWDGE rings (sync + scalar);
           Q^T/K^T/V projections for seq chunk 0 pipelined per d-chunk.
  stage j  attention for q-chunk j (512 q rows x all k-blocks <= diag):
             S^T pair [128k, 2x512q] psum (2 banks): both heads' QK^T
               matmuls emitted adjacently with tile_position row packing so
               they run CONCURRENTLY in the PE array (64-contraction each).
             exp on ACT as ONE [128, 2, 512-rs] instruction per k-block
               (both heads), bf16 out; fully-masked leading cols skipped,
               diagonal blocks get a cmask add (DVE) pre-exp.
             AV accumulated over k-blocks into [72, 512] psum per head;
               col 64 of V = ones => row 64 = softmax denominators.
           Interleaved as PE filler: projections for chunk j+1 (stages 0-2)
           and the output projection for chunks 0..2 (stage 3), so the PE
           stream stays dense while ACT works through the exps.
  norm     reciprocal_approx_fast on the denominators (5x faster than the
           iterative divide), gpsimd partition_broadcast, DVE multiply.
  out      OUT [128s, 512c] = O^T.T @ Wo accumulated over 4 head-pair
           chunks; last chunk's final head-pair contribution added
           separately so the tail doesn't serialize.
"""

import itertools
import sys

for _p in ("/opt/trn_rl_repo", "/root/.axon_site/_ro/trn_rl_repo"):
    if _p not in sys.path:
        sys.path.insert(0, _p)

import ml_dtypes
import numpy as np

import concourse.bass as bass
import concourse.mybir as mybir
import concourse.tile as tile
from concourse import bacc
from concourse.bass_utils import run_bass_kernel_spmd

F32 = mybir.dt.float32
BF16 = mybir.dt.bfloat16
EXPF = mybir.ActivationFunctionType.Exp

B, S, D, H = 4, 2048, 1024, 16
HD = D // H           # 64
HL = H // 2           # 8 heads per core
DL = HL * HD          # 512 local proj width
NEG = -30000.0        # causal mask additive value (exp underflows to 0)
VW = 72               # AV lhsT width: 64 V cols + ones col + 7 pad


class _Filler:
    """Interleave a generator of PE work quanta at a fractional rate."""

    def __init__(self, gens):
        self.it = itertools.chain(*gens)
        self.frac = 0.0
        self.done = False

    def pump(self, amount):
        if self.done:
            return
        self.frac += amount
        while self.frac >= 1.0:
            try:
                next(self.it)
            except StopIteration:
                self.done = True
                return
            self.frac -= 1.0

    def drain(self):
        for _ in self.it:
            pass
        self.done = True


def build_program(s=S, d=D, hl=HL):
    dl = hl * HD
    n_st = s // 128          # 16 s-tiles (128 rows)
    n_dc = d // 128          # 8 d-chunks (projection contraction)
    n_pc = dl // 128         # 4 head-pair chunks
    n_q = s // 512           # 4 q-chunks
    n_k = s // 128           # 16 k-blocks
    n_cc = d // 512          # 2 out column chunks

    nc = bacc.Bacc("TRN2", target_bir_lowering=False, debug=False)

    X = nc.dram_tensor("X", [s, d], BF16, kind="ExternalInput")
    WQ = nc.dram_tensor("WQ", [d, dl], BF16, kind="ExternalInput")
    WK = nc.dram_tensor("WK", [d, dl], BF16, kind="ExternalInput")
    WV = nc.dram_tensor("WV", [d, dl], BF16, kind="ExternalInput")
    WO = nc.dram_tensor("WO", [dl, d], BF16, kind="ExternalInput")
    OUT = nc.dram_tensor("OUT", [s, d], F32, kind="ExternalOutput")
    # last head-pair's contribution to the last seq chunk, summed on host
    # (avoids serializing the tail on an on-chip add)
    OUT2 = nc.dram_tensor("OUT2", [512, d], F32, kind="ExternalOutput")

    with tile.TileContext(nc) as tc:
        with tc.tile_pool(name="persist", bufs=1) as persist:
            # diagonal causal mask block x2 (keep where q >= k), one copy
            # per head so a single DVE add masks both heads' diag blocks
            cmask2 = persist.tile([128, 2, 128], F32, name="cmask2")
            nc.gpsimd.memset(cmask2[:], 0.0)
            for hb in (0, 1):
                nc.gpsimd.affine_select(
                    out=cmask2[:, hb, :], in_=cmask2[:, hb, :],
                    compare_op=mybir.AluOpType.is_ge, fill=NEG,
                    base=0, pattern=[[1, 128]], channel_multiplier=-1,
                )

            # X^T in chunk-major layout: xt[p, nq, dc, m] = X^T[dc*128+p,
            # nq*512+m]. Each seq-quarter of X is one CONTIGUOUS DMA
            # transpose writing one contiguous SBUF region — DMA transposes
            # serialize globally against all other DMAs (HW deadlock guard),
            # so fewer/bigger transposes shorten the ramp chain.
            xt = persist.tile([128, n_q, n_dc, 512], BF16, name="xt")
            qt = [persist.tile([128, s], BF16, name=f"qt{i}") for i in range(n_pc)]
            kt = [persist.tile([128, s], BF16, name=f"kt{i}") for i in range(n_pc)]
            vt = [persist.tile([128, hl, VW], BF16, name=f"vt{i}") for i in range(n_st)]
            ot = [persist.tile([128, s], BF16, name=f"ot{i}") for i in range(n_pc)]
            wq = persist.tile([128, n_dc, dl], BF16, name="wq")
            wk = persist.tile([128, n_dc, dl], BF16, name="wk")
            wv = persist.tile([128, n_dc, dl], BF16, name="wv")
            wo = persist.tile([128, n_pc, d], BF16, name="wo")

            # DMA kickoff: ALL on the sync ring in dependency order. DMA
            # transposes serialize globally against in-flight DMAs (HW
            # deadlock guard); spreading across rings just ping-pongs with
            # a multi-us completion-latency hop each time.
            nc.sync.dma_start(
                wq[:], WQ.ap().rearrange("(c p) m -> p c m", p=128))
            nc.sync.dma_start(
                wk[:], WK.ap().rearrange("(c p) m -> p c m", p=128))
            nc.sync.dma_start(
                wv[:], WV.ap().rearrange("(c p) m -> p c m", p=128))
            for nq in range(n_q):
                nc.sync.dma_start(
                    xt[:, nq], X[nq * 512:(nq + 1) * 512, :], transpose=True)
            nc.sync.dma_start(
                wo[:], WO.ap().rearrange("(c p) m -> p c m", p=128))

            with (
                tc.tile_pool(name="ppp", bufs=2, space="PSUM") as ppp,
                tc.tile_pool(name="stpp", bufs=2, space="PSUM") as stpp,
                tc.tile_pool(name="avp", bufs=2, space="PSUM") as avp,
                tc.tile_pool(name="work", bufs=4) as work,
                tc.tile_pool(name="osbp", bufs=8) as osbp,
            ):
                def gen_proj(nq):
                    """Projection of seq chunk nq; yields per PE quantum.
                    Order Q-pc, K-pc, V-st interleaved so the first
                    attention unit's inputs land earliest."""
                    for pc in range(n_pc):
                        for w, dst, cpy in ((wq, qt, "act"), (wk, kt, "dve")):
                            ps = ppp.tile([128, 512], F32, tag="pp",
                                          name=f"psp{nq}_{pc}")
                            for dc in range(n_dc):
                                nc.tensor.matmul(
                                    ps[:], w[:, dc, pc * 128:(pc + 1) * 128],
                                    xt[:, nq, dc, :],
                                    start=(dc == 0), stop=(dc == n_dc - 1))
                                yield
                            qs = slice(nq * 512, (nq + 1) * 512)
                            if cpy == "act":
                                nc.scalar.copy(dst[pc][:, qs], ps[:])
                            else:
                                nc.vector.tensor_copy(dst[pc][:, qs], ps[:])
                            yield
                        st = 4 * nq + pc
                        ps = ppp.tile([128, dl], F32, tag="pp",
                                      name=f"psv{nq}_{st}")
                        for dc in range(n_dc):
                            nc.tensor.matmul(
                                ps[:], xt[:, nq, dc, pc * 128:(pc + 1) * 128],
                                wv[:, dc, :],
                                start=(dc == 0), stop=(dc == n_dc - 1))
                            yield
                        nc.gpsimd.memset(vt[st][:], 1.0)
                        nc.vector.tensor_copy(
                            vt[st][:, :, 0:64],
                            ps[:].rearrange("p (h e) -> p h e", h=hl))
                        yield

                def gen_outproj(j, skip_last_pc=False):
                    """Output projection for seq chunk j. With skip_last_pc,
                    only head-pairs 0..n_pc-2 are accumulated and written to
                    OUT; the last pair goes to OUT2 via the finisher once
                    its normalize lands (summed on the host)."""
                    npc = n_pc - 1 if skip_last_pc else n_pc
                    for st in range(4 * j, 4 * j + 4):
                        for cc in range(n_cc):
                            ps = ppp.tile([128, 512], F32, tag="pp",
                                          name=f"pso{st}_{cc}")
                            for pc in range(npc):
                                nc.tensor.matmul(
                                    ps[:], ot[pc][:, st * 128:(st + 1) * 128],
                                    wo[:, pc, cc * 512:(cc + 1) * 512],
                                    start=(pc == 0), stop=(pc == npc - 1))
                                yield
                            osb = osbp.tile([128, 512], F32, tag="osb",
                                            name=f"osb{st}_{cc}")
                            nc.vector.tensor_copy(osb[:], ps[:])
                            yield
                            # alternate rings so OUT writes don't back up
                            eng = nc.sync if (st + cc) % 2 == 0 else nc.scalar
                            eng.dma_start(
                                OUT[st * 128:(st + 1) * 128,
                                    cc * 512:(cc + 1) * 512],
                                osb[:])
                            yield

                def attn_unit(j, pc, fillers):
                    js = slice(j * 512, (j + 1) * 512)
                    n_i = min(4 * j + 4, n_k)
                    av = [avp.tile([VW, 512], F32, tag="av",
                                   name=f"av{j}_{pc}_{h}") for h in (0, 1)]
                    for i in range(n_i):
                        r = i - 4 * j
                        rs = max(r, 0) * 128   # fully-masked leading cols
                        stp = stpp.tile([128, 2, 512], F32, tag="stp",
                                        name=f"stp{j}_{pc}_{i}")
                        for h in (0, 1):
                            hs = slice(64 * h, 64 * h + 64)
                            nc.tensor.matmul(
                                stp[:, h, rs:512],
                                kt[pc][hs, i * 128:(i + 1) * 128],
                                qt[pc][hs, j * 512 + rs:(j + 1) * 512],
                                start=True, stop=True,
                                tile_position=(64 * h, 0))
                        if r >= 0:
                            nc.vector.tensor_add(
                                stp[:, :, rs:rs + 128], stp[:, :, rs:rs + 128],
                                cmask2[:])
                        et = work.tile([128, 2, 512], BF16, tag="et", bufs=6,
                                       name=f"et{j}_{pc}_{i}")
                        nc.scalar.activation(
                            et[:, :, rs:512], stp[:, :, rs:512], EXPF,
                            scale=0.125)
                        # half the filler between exp and AV: the AV matmuls
                        # wait on exp completion (~0.6us) — give the PE
                        # ready work at exactly that point in priority order
                        for flr, rate in fillers:
                            flr.pump(rate / 2)
                        for h in (0, 1):
                            nc.tensor.matmul(
                                av[h][:, rs:512], vt[i][:, 2 * pc + h, :],
                                et[:, h, rs:512],
                                start=(i == 0), stop=(i == n_i - 1))
                        for flr, rate in fillers:
                            flr.pump(rate / 2)

                    # normalize: denominators live in av row 64. Per-head
                    # independent chains: copy O'+denom to SBUF, DMA the
                    # denom row to partition 0, approx-reciprocal, gpsimd
                    # broadcast, DVE multiply (h1 DMA-shifts to rows 64-127).
                    for h in (0, 1):
                        orw = work.tile([VW, 512], F32, tag="orw", bufs=4,
                                        name=f"orw{j}_{pc}_{h}")
                        nc.vector.tensor_copy(orw[:], av[h][:])
                        dgp = work.tile([1, 512], F32, tag=f"dg{h}", bufs=3,
                                        name=f"dg{j}_{pc}_{h}")
                        nc.sync.dma_start(dgp[:], orw[64:65, :])
                        rgp = work.tile([1, 512], F32, tag=f"rg{h}", bufs=3,
                                        name=f"rg{j}_{pc}_{h}")
                        nc.vector.reciprocal_approx_fast(rgp[:], dgp[:])
                        bc = work.tile([64, 512], F32, tag=f"bc{h}", bufs=3,
                                       name=f"bc{j}_{pc}_{h}")
                        nc.gpsimd.partition_broadcast(bc[:], rgp[:])
                        if h == 0:
                            nc.vector.tensor_mul(
                                ot[pc][0:64, js], orw[0:64, :], bc[:])
                        else:
                            sc = work.tile([64, 512], BF16, tag="sc", bufs=3,
                                           name=f"sc{j}_{pc}")
                            nc.vector.tensor_mul(sc[:], orw[0:64, :], bc[:])
                            nc.sync.dma_start(ot[pc][64:128, js], sc[:])

                # ---- ramp: projections for chunk 0, dc-pipelined ----
                for _ in gen_proj(0):
                    pass

                # ---- pipelined stages ----
                for j in range(n_q):
                    if j < n_q - 1:
                        filler = _Filler([gen_proj(j + 1)])
                        rate = {0: 7.0, 1: 3.6, 2: 2.4}[j]
                    else:
                        # outproj(2) is held back past the last unit so the
                        # PE has work while the final normalize drains
                        filler = _Filler([gen_outproj(0), gen_outproj(1)])
                        rate = 1.6
                    for pc in range(n_pc):
                        fillers = [(filler, rate)]
                        if j == n_q - 1 and pc == n_pc - 1:
                            part1 = _Filler([gen_outproj(3, skip_last_pc=True)])
                            fillers.append((part1, 2.0))
                        attn_unit(j, pc, fillers)
                        if j == n_q - 1 and pc == n_pc - 1:
                            part1.drain()
                    filler.drain()
                for _ in gen_outproj(2):
                    pass

                # ---- finisher: last head-pair x chunk 3 -> OUT2 ----
                for st in range(4 * (n_q - 1), 4 * n_q):
                    for cc in range(n_cc):
                        psb = ppp.tile([128, 512], F32, tag="pp",
                                       name=f"psb{st}_{cc}")
                        nc.tensor.matmul(
                            psb[:], ot[n_pc - 1][:, st * 128:(st + 1) * 128],
                            wo[:, n_pc - 1, cc * 512:(cc + 1) * 512],
                            start=True, stop=True)
                        osb = osbp.tile([128, 512], F32, tag="osb",
                                        name=f"osb2{st}_{cc}")
                        nc.vector.tensor_copy(osb[:], psb[:])
                        eng = nc.sync if (st + cc) % 2 == 0 else nc.scalar
                        eng.dma_start(
                            OUT2[(st - 4 * (n_q - 1)) * 128:
                                 (st - 4 * (n_q - 1) + 1) * 128,
                                 cc * 512:(cc + 1) * 512],
                            osb[:])

    nc.compile()
    return nc


_NC_CACHE = {}


def _get_program():
    key = (S, D, HL)
    if key not in _NC_CACHE:
        _NC_CACHE[key] = build_program()
    return _NC_CACHE[key]


def _bf16(a):
    return np.ascontiguousarray(a.astype(ml_dtypes.bfloat16))


def make_in_maps(X, Wq, Wk, Wv, Wo):
    in_maps = []
    for c in range(8):
        b, hg = c // 2, c % 2
        cs = slice(hg * DL, hg * DL + DL)
        in_maps.append({
            "X": _bf16(X[b]),
            "WQ": _bf16(Wq[:, cs]),
            "WK": _bf16(Wk[:, cs]),
            "WV": _bf16(Wv[:, cs]),
            "WO": _bf16(Wo[cs, :]),
        })
    return in_maps


def gather_out(results):
    out = np.empty((B, S, D), dtype=np.float32)
    for b in range(B):
        out[b] = results[2 * b]["OUT"] + results[2 * b + 1]["OUT"]
        out[b, S - 512:] += results[2 * b]["OUT2"] + results[2 * b + 1]["OUT2"]
    return out


def kernel(X, Wq, Wk, Wv, Wo):
    X = np.asarray(X, dtype=np.float32)
    Wq = np.asarray(Wq, dtype=np.float32)
    Wk = np.asarray(Wk, dtype=np.float32)
    Wv = np.asarray(Wv, dtype=np.float32)
    Wo = np.asarray(Wo, dtype=np.float32)

    nc = _get_program()
    in_maps = make_in_maps(X, Wq, Wk, Wv, Wo)
    res = run_bass_kernel_spmd(nc, in_maps, list(range(8)), trace=False)
    return gather_out(res.results)


if __name__ == "__main__":
    rng = np.random.default_rng(0)
    scale = 1.0 / np.sqrt(D)
    inputs = {
        "X": rng.standard_normal((B, S, D), dtype=np.float32),
        "Wq": rng.standard_normal((D, D), dtype=np.float32) * scale,
        "Wk": rng.standard_normal((D, D), dtype=np.float32) * scale,
        "Wv": rng.standard_normal((D, D), dtype=np.float32) * scale,
        "Wo": rng.standard_normal((D, D), dtype=np.float32) * scale,
    }
    out = kernel(**inputs)
    print("kernel output shape:", out.shape)


# revision 19
# speedup vs baseline: 1.4083x; 1.0079x over previous
"""Trainium2 Bass kernel for multi-head causal self-attention.

Problem: X [4, 2048, 1024] fp32, Wq/Wk/Wv/Wo [1024, 1024], H=16 heads, HD=64.
reference: out = softmax_causal((X@Wq) (X@Wk)^T / 8) (X@Wv) merged @ Wo.

Sharding over 8 NeuronCores: core c handles batch b = c // 2 and head group
hg = c % 2 (8 heads each). Each core computes a partial [2048, 1024] output
(its heads' contribution through Wo's row shard); the host sums the two
partials per batch (the tensor-parallel all-reduce, done during unsharding).

Per-core dataflow (bf16 operands, fp32 PSUM accumulation), software-pipelined
so the PE never starves (keeps the HAM clock-gate warm):

  ramp     X^T via DMA-transpose on BOTH HWDGE rings (sync + scalar);
           Q^T/K^T/V projections for seq chunk 0 pipelined per d-chunk.
  stage j  attention for q-chunk j (512 q rows x all k-blocks <= diag):
             S^T pair [128k, 2x512q] psum (2 banks): both heads' QK^T
               matmuls emitted adjacently with tile_position row packing so
               they run CONCURRENTLY in the PE array (64-contraction each).
             exp on ACT as ONE [128, 2, 512-rs] instruction per k-block
               (both heads), bf16 out; fully-masked leading cols skipped,
               diagonal blocks get a cmask add (DVE) pre-exp.
             AV accumulated over k-blocks into [72, 512] psum per head;
               col 64 of V = ones => row 64 = softmax denominators.
           Interleaved as PE filler: projections for chunk j+1 (stages 0-2)
           and the output projection for chunks 0..2 (stage 3), so the PE
           stream stays dense while ACT works through the exps.
  norm     reciprocal_approx_fast on the denominators (5x faster than the
           iterative divide), gpsimd partition_broadcast, DVE multiply.
  out      OUT [128s, 512c] = O^T.T @ Wo accumulated over 4 head-pair
           chunks; last chunk's final head-pair contribution added
           separately so the tail doesn't serialize.
"""

import itertools
import sys

for _p in ("/opt/trn_rl_repo", "/root/.axon_site/_ro/trn_rl_repo"):
    if _p not in sys.path:
        sys.path.insert(0, _p)

import ml_dtypes
import numpy as np

import concourse.bass as bass
import concourse.mybir as mybir
import concourse.tile as tile
from concourse import bacc
from concourse.bass_utils import run_bass_kernel_spmd

F32 = mybir.dt.float32
BF16 = mybir.dt.bfloat16
EXPF = mybir.ActivationFunctionType.Exp

B, S, D, H = 4, 2048, 1024, 16
HD = D // H           # 64
HL = H // 2           # 8 heads per core
DL = HL * HD          # 512 local proj width
NEG = -30000.0        # causal mask additive value (exp underflows to 0)
VW = 72               # AV lhsT width: 64 V cols + ones col + 7 pad


class _Filler:
    """Interleave a generator of PE work quanta at a fractional rate."""

    def __init__(self, gens):
        self.it = itertools.chain(*gens)
        self.frac = 0.0
        self.done = False

    def pump(self, amount):
        if self.done:
            return
        self.frac += amount
        while self.frac >= 1.0:
            try:
                next(self.it)
            except StopIteration:
                self.done = True
                return
            self.frac -= 1.0

    def drain(self):
        for _ in self.it:
            pass
        self.done = True


def build_program(s=S, d=D, hl=HL):
    dl = hl * HD
    n_st = s // 128          # 16 s-tiles (128 rows)
    n_dc = d // 128          # 8 d-chunks (projection contraction)
    n_pc = dl // 128         # 4 head-pair chunks
    n_q = s // 512           # 4 q-chunks
    n_k = s // 128           # 16 k-blocks
    n_cc = d // 512          # 2 out column chunks

    nc = bacc.Bacc("TRN2", target_bir_lowering=False, debug=False)

    X = nc.dram_tensor("X", [s, d], BF16, kind="ExternalInput")
    WQ = nc.dram_tensor("WQ", [d, dl], BF16, kind="ExternalInput")
    WK = nc.dram_tensor("WK", [d, dl], BF16, kind="ExternalInput")
    WV = nc.dram_tensor("WV", [d, dl], BF16, kind="ExternalInput")
    WO = nc.dram_tensor("WO", [dl, d], BF16, kind="ExternalInput")
    OUT = nc.dram_tensor("OUT", [s, d], BF16, kind="ExternalOutput")
    # last head-pair's contribution to the last seq chunk, summed on host
    # (avoids serializing the tail on an on-chip add)
    OUT2 = nc.dram_tensor("OUT2", [512, d], BF16, kind="ExternalOutput")

    with tile.TileContext(nc) as tc:
        with tc.tile_pool(name="persist", bufs=1) as persist:
            # diagonal causal mask block x2 (keep where q >= k), one copy
            # per head so a single DVE add masks both heads' diag blocks
            cmask2 = persist.tile([128, 2, 128], F32, name="cmask2")
            nc.gpsimd.memset(cmask2[:], 0.0)
            for hb in (0, 1):
                nc.gpsimd.affine_select(
                    out=cmask2[:, hb, :], in_=cmask2[:, hb, :],
                    compare_op=mybir.AluOpType.is_ge, fill=NEG,
                    base=0, pattern=[[1, 128]], channel_multiplier=-1,
                )

            # X^T in chunk-major layout: xt[p, nq, dc, m] = X^T[dc*128+p,
            # nq*512+m]. Each seq-quarter of X is one CONTIGUOUS DMA
            # transpose writing one contiguous SBUF region — DMA transposes
            # serialize globally against all other DMAs (HW deadlock guard),
            # so fewer/bigger transposes shorten the ramp chain.
            xt = persist.tile([128, n_q, n_dc, 512], BF16, name="xt")
            qt = [persist.tile([128, s], BF16, name=f"qt{i}") for i in range(n_pc)]
            kt = [persist.tile([128, s], BF16, name=f"kt{i}") for i in range(n_pc)]
            vt = [persist.tile([128, hl, VW], BF16, name=f"vt{i}") for i in range(n_st)]
            ot = [persist.tile([128, s], BF16, name=f"ot{i}") for i in range(n_pc)]
            wq = persist.tile([128, n_dc, dl], BF16, name="wq")
            wk = persist.tile([128, n_dc, dl], BF16, name="wk")
            wv = persist.tile([128, n_dc, dl], BF16, name="wv")
            wo = persist.tile([128, n_pc, d], BF16, name="wo")

            # DMA kickoff: ALL on the sync ring in dependency order. DMA
            # transposes serialize globally against in-flight DMAs (HW
            # deadlock guard); every plain-DMA/transpose alternation pays
            # a multi-us completion-latency hop, so weights go first in
            # one batch, then the four transposes back-to-back.
            nc.sync.dma_start(
                wq[:], WQ.ap().rearrange("(c p) m -> p c m", p=128))
            nc.sync.dma_start(
                wk[:], WK.ap().rearrange("(c p) m -> p c m", p=128))
            nc.sync.dma_start(
                wv[:], WV.ap().rearrange("(c p) m -> p c m", p=128))
            nc.sync.dma_start(
                wo[:], WO.ap().rearrange("(c p) m -> p c m", p=128))
            for nq in range(n_q):
                nc.sync.dma_start(
                    xt[:, nq], X[nq * 512:(nq + 1) * 512, :], transpose=True)

            with (
                tc.tile_pool(name="ppp", bufs=2, space="PSUM") as ppp,
                tc.tile_pool(name="stpp", bufs=2, space="PSUM") as stpp,
                tc.tile_pool(name="avp", bufs=2, space="PSUM") as avp,
                tc.tile_pool(name="work", bufs=4) as work,
                tc.tile_pool(name="osbp", bufs=8) as osbp,
            ):
                def gen_proj(nq):
                    """Projection of seq chunk nq; yields per PE quantum.
                    Order Q-pc, K-pc, V-st interleaved so the first
                    attention unit's inputs land earliest."""
                    for pc in range(n_pc):
                        for w, dst, cpy in ((wq, qt, "act"), (wk, kt, "dve")):
                            ps = ppp.tile([128, 512], F32, tag="pp",
                                          name=f"psp{nq}_{pc}")
                            for dc in range(n_dc):
                                nc.tensor.matmul(
                                    ps[:], w[:, dc, pc * 128:(pc + 1) * 128],
                                    xt[:, nq, dc, :],
                                    start=(dc == 0), stop=(dc == n_dc - 1))
                                yield
                            qs = slice(nq * 512, (nq + 1) * 512)
                            if cpy == "act":
                                nc.scalar.copy(dst[pc][:, qs], ps[:])
                            else:
                                nc.vector.tensor_copy(dst[pc][:, qs], ps[:])
                            yield
                        st = 4 * nq + pc
                        ps = ppp.tile([128, dl], F32, tag="pp",
                                      name=f"psv{nq}_{st}")
                        for dc in range(n_dc):
                            nc.tensor.matmul(
                                ps[:], xt[:, nq, dc, pc * 128:(pc + 1) * 128],
                                wv[:, dc, :],
                                start=(dc == 0), stop=(dc == n_dc - 1))
                            yield
                        nc.gpsimd.memset(vt[st][:], 1.0)
                        nc.vector.tensor_copy(
                            vt[st][:, :, 0:64],
                            ps[:].rearrange("p (h e) -> p h e", h=hl))
                        yield

                def gen_outproj(j, skip_last_pc=False):
                    """Output projection for seq chunk j. With skip_last_pc,
                    only head-pairs 0..n_pc-2 are accumulated and written to
                    OUT; the last pair goes to OUT2 via the finisher once
                    its normalize lands (summed on the host)."""
                    npc = n_pc - 1 if skip_last_pc else n_pc
                    for st in range(4 * j, 4 * j + 4):
                        for cc in range(n_cc):
                            ps = ppp.tile([128, 512], F32, tag="pp",
                                          name=f"pso{st}_{cc}")
                            for pc in range(npc):
                                nc.tensor.matmul(
                                    ps[:], ot[pc][:, st * 128:(st + 1) * 128],
                                    wo[:, pc, cc * 512:(cc + 1) * 512],
                                    start=(pc == 0), stop=(pc == npc - 1))
                                yield
                            osb = osbp.tile([128, 512], BF16, tag="osb",
                                            name=f"osb{st}_{cc}")
                            nc.vector.tensor_copy(osb[:], ps[:])
                            yield
                            # alternate rings so OUT writes don't back up
                            eng = nc.sync if (st + cc) % 2 == 0 else nc.scalar
                            eng.dma_start(
                                OUT[st * 128:(st + 1) * 128,
                                    cc * 512:(cc + 1) * 512],
                                osb[:])
                            yield

                def attn_unit(j, pc, fillers):
                    js = slice(j * 512, (j + 1) * 512)
                    n_i = min(4 * j + 4, n_k)
                    av = [avp.tile([VW, 512], F32, tag="av",
                                   name=f"av{j}_{pc}_{h}") for h in (0, 1)]
                    for i in range(n_i):
                        r = i - 4 * j
                        rs = max(r, 0) * 128   # fully-masked leading cols
                        stp = stpp.tile([128, 2, 512], F32, tag="stp",
                                        name=f"stp{j}_{pc}_{i}")
                        for h in (0, 1):
                            hs = slice(64 * h, 64 * h + 64)
                            nc.tensor.matmul(
                                stp[:, h, rs:512],
                                kt[pc][hs, i * 128:(i + 1) * 128],
                                qt[pc][hs, j * 512 + rs:(j + 1) * 512],
                                start=True, stop=True,
                                tile_position=(64 * h, 0))
                        if r >= 0:
                            nc.vector.tensor_add(
                                stp[:, :, rs:rs + 128], stp[:, :, rs:rs + 128],
                                cmask2[:])
                        et = work.tile([128, 2, 512], BF16, tag="et", bufs=6,
                                       name=f"et{j}_{pc}_{i}")
                        nc.scalar.activation(
                            et[:, :, rs:512], stp[:, :, rs:512], EXPF,
                            scale=0.125)
                        # half the filler between exp and AV: the AV matmuls
                        # wait on exp completion (~0.6us) — give the PE
                        # ready work at exactly that point in priority order
                        for flr, rate in fillers:
                            flr.pump(rate / 2)
                        for h in (0, 1):
                            nc.tensor.matmul(
                                av[h][:, rs:512], vt[i][:, 2 * pc + h, :],
                                et[:, h, rs:512],
                                start=(i == 0), stop=(i == n_i - 1))
                        for flr, rate in fillers:
                            flr.pump(rate / 2)

                    # normalize: denominators live in av row 64. Per-head
                    # independent chains: copy O'+denom to SBUF, DMA the
                    # denom row to partition 0, approx-reciprocal, gpsimd
                    # broadcast, DVE multiply (h1 DMA-shifts to rows 64-127).
                    for h in (0, 1):
                        orw = work.tile([VW, 512], F32, tag="orw", bufs=4,
                                        name=f"orw{j}_{pc}_{h}")
                        nc.vector.tensor_copy(orw[:], av[h][:])
                        dgp = work.tile([1, 512], F32, tag=f"dg{h}", bufs=3,
                                        name=f"dg{j}_{pc}_{h}")
                        nc.sync.dma_start(dgp[:], orw[64:65, :])
                        rgp = work.tile([1, 512], F32, tag=f"rg{h}", bufs=3,
                                        name=f"rg{j}_{pc}_{h}")
                        nc.vector.reciprocal_approx_fast(rgp[:], dgp[:])
                        bc = work.tile([64, 512], F32, tag=f"bc{h}", bufs=3,
                                       name=f"bc{j}_{pc}_{h}")
                        nc.gpsimd.partition_broadcast(bc[:], rgp[:])
                        if h == 0:
                            nc.vector.tensor_mul(
                                ot[pc][0:64, js], orw[0:64, :], bc[:])
                        else:
                            sc = work.tile([64, 512], BF16, tag="sc", bufs=3,
                                           name=f"sc{j}_{pc}")
                            nc.vector.tensor_mul(sc[:], orw[0:64, :], bc[:])
                            nc.sync.dma_start(ot[pc][64:128, js], sc[:])

                # ---- ramp: projections for chunk 0, dc-pipelined ----
                for _ in gen_proj(0):
                    pass

                # ---- pipelined stages ----
                for j in range(n_q):
                    if j < n_q - 1:
                        filler = _Filler([gen_proj(j + 1)])
                        rate = {0: 7.0, 1: 3.6, 2: 2.4}[j]
                    else:
                        filler = _Filler([gen_outproj(0), gen_outproj(1),
                                          gen_outproj(2)])
                        rate = 2.2
                    for pc in range(n_pc):
                        fillers = [(filler, rate)]
                        if j == n_q - 1 and pc == n_pc - 1:
                            part1 = _Filler([gen_outproj(3, skip_last_pc=True)])
                            fillers.append((part1, 2.0))
                        attn_unit(j, pc, fillers)
                        if j == n_q - 1 and pc == n_pc - 1:
                            part1.drain()
                    filler.drain()

                # ---- finisher: last head-pair x chunk 3 -> OUT2 ----
                for st in range(4 * (n_q - 1), 4 * n_q):
                    for cc in range(n_cc):
                        psb = ppp.tile([128, 512], F32, tag="pp",
                                       name=f"psb{st}_{cc}")
                        nc.tensor.matmul(
                            psb[:], ot[n_pc - 1][:, st * 128:(st + 1) * 128],
                            wo[:, n_pc - 1, cc * 512:(cc + 1) * 512],
                            start=True, stop=True)
                        osb = osbp.tile([128, 512], BF16, tag="osb",
                                        name=f"osb2{st}_{cc}")
                        nc.vector.tensor_copy(osb[:], psb[:])
                        eng = nc.sync if (st + cc) % 2 == 0 else nc.scalar
                        eng.dma_start(
                            OUT2[(st - 4 * (n_q - 1)) * 128:
                                 (st - 4 * (n_q - 1) + 1) * 128,
                                 cc * 512:(cc + 1) * 512],
                            osb[:])

    nc.compile()
    return nc


_NC_CACHE = {}


def _get_program():
    key = (S, D, HL)
    if key not in _NC_CACHE:
        _NC_CACHE[key] = build_program()
    return _NC_CACHE[key]


def _bf16(a):
    return np.ascontiguousarray(a.astype(ml_dtypes.bfloat16))


def make_in_maps(X, Wq, Wk, Wv, Wo):
    in_maps = []
    for c in range(8):
        b, hg = c // 2, c % 2
        cs = slice(hg * DL, hg * DL + DL)
        in_maps.append({
            "X": _bf16(X[b]),
            "WQ": _bf16(Wq[:, cs]),
            "WK": _bf16(Wk[:, cs]),
            "WV": _bf16(Wv[:, cs]),
            "WO": _bf16(Wo[cs, :]),
        })
    return in_maps


def gather_out(results):
    out = np.empty((B, S, D), dtype=np.float32)
    for b in range(B):
        out[b] = (results[2 * b]["OUT"].astype(np.float32)
                  + results[2 * b + 1]["OUT"].astype(np.float32))
        out[b, S - 512:] += (results[2 * b]["OUT2"].astype(np.float32)
                             + results[2 * b + 1]["OUT2"].astype(np.float32))
    return out


def kernel(X, Wq, Wk, Wv, Wo):
    X = np.asarray(X, dtype=np.float32)
    Wq = np.asarray(Wq, dtype=np.float32)
    Wk = np.asarray(Wk, dtype=np.float32)
    Wv = np.asarray(Wv, dtype=np.float32)
    Wo = np.asarray(Wo, dtype=np.float32)

    nc = _get_program()
    in_maps = make_in_maps(X, Wq, Wk, Wv, Wo)
    res = run_bass_kernel_spmd(nc, in_maps, list(range(8)), trace=False)
    return gather_out(res.results)


if __name__ == "__main__":
    rng = np.random.default_rng(0)
    scale = 1.0 / np.sqrt(D)
    inputs = {
        "X": rng.standard_normal((B, S, D), dtype=np.float32),
        "Wq": rng.standard_normal((D, D), dtype=np.float32) * scale,
        "Wk": rng.standard_normal((D, D), dtype=np.float32) * scale,
        "Wv": rng.standard_normal((D, D), dtype=np.float32) * scale,
        "Wo": rng.standard_normal((D, D), dtype=np.float32) * scale,
    }
    out = kernel(**inputs)
    print("kernel output shape:", out.shape)
